# revision 16
# baseline (speedup 1.0000x reference)
"""BCQLinear (3-bit binary-coded quantized linear) Trainium2 kernel.

Full-input contract: kernel(**inputs) takes the unsharded inputs of
nn_BCQLinear_88510686036218 and returns the full [1, 128, 4096] output.

Math: w = alpha*(2*S-7) + beta with S in [0,8) the 3-bit code, then
y = (x[:, in_reorder] @ w)[:, out_reorder].
Rewritten: w = A2*V + B with V = S-4 in [-4,4), A2 = 2*alpha, B = alpha+beta.

Sharding: out-features split 8 ways (512 cols/core), x replicated.

Per-core device program (SPMD, one Bass program):
  - Contraction rows are band-packed: K-tile kt in [0,32), partition p:
    dequant row i(kt,p) = 128*(p//4) + 4*kt + (p%4), so a single [128,512]
    alpha tile (A2_rep[p,:] = A2[p//4,:]) serves every K-tile.
  - Codes arrive as packed int32 words: 8 fields of 3 bits at bits
    [29-3r, 32-3r), biased so field value F = (S+4)&7 decodes to V = S-4
    via one two-op tensor_scalar: V = (W << 3r) >>(arith) 29.
  - Work is split into NCHUNK K-tile chunks so DMA / unpack (DVE) /
    dequant-mult (DVE) / matmul (PE) pipeline across chunks.
  - The beta part is y += xsum @ B with xsum[g,t] the per-group sums of
    permuted x (host-computed; 0.01% of the FLOPs) via one K=32 matmul.
"""
import numpy as np
from contextlib import ExitStack

import concourse.bass as bass
import concourse.mybir as mybir
import concourse.tile as tile
from concourse import bacc

IN_F, OUT_F, WBITS, GS, OFI = 4096, 4096, 3, 128, 128
NG, NB = 32, 32
NCORES = 8
OPC = OUT_F // NCORES        # 512 out-cols per core
NKT = 32                     # K-tiles of 128 rows
NF = 8                       # 3-bit fields packed per int32 word
NWC = OPC // NF              # 64 words per (partition, K-tile)
T = 128                      # tokens
NCHUNK = 2                   # pipeline chunks
KTC = NKT // NCHUNK          # K-tiles per chunk

F32 = mybir.dt.float32
I32 = mybir.dt.int32
ALU = mybir.AluOpType

_PROGRAM_CACHE = {}


# ---------------------------------------------------------------- host prep
def _dequant_codes(qweight):
    """S[i, o] in [0,8): w = alpha*(2S-7)+beta."""
    qw = np.asarray(qweight, dtype=np.uint32).reshape(NG, NB, WBITS, GS * OFI // 32)
    bits = (qw[..., None] >> np.arange(32, dtype=np.uint32)) & 1
    bits = bits.reshape(NG, NB, WBITS, GS, OFI)
    S = (bits * (2 ** np.arange(WBITS, dtype=np.uint32))[:, None, None]).sum(axis=2)
    return S.transpose(0, 2, 1, 3).reshape(IN_F, OUT_F).astype(np.int32)


def _band_rows():
    kt, p = np.meshgrid(np.arange(NKT), np.arange(128), indexing="ij")
    return 128 * (p // 4) + 4 * kt + (p % 4)      # [NKT, 128]


def _prepare(inputs):
    x = np.asarray(inputs["x"], np.float32).reshape(-1, IN_F)
    alpha = np.asarray(inputs["alpha"], np.float32)
    beta = np.asarray(inputs["beta"], np.float32)
    in_reorder = np.asarray(inputs["in_reorder"], np.int64)
    xf = x[:, in_reorder]

    S = _dequant_codes(inputs["qweight"])
    A2full = (2.0 * alpha).astype(np.float32)
    Bfull = (alpha + beta).astype(np.float32)

    rows = _band_rows()                            # [NKT, 128]
    XT = np.ascontiguousarray(
        xf[:, rows.reshape(-1)].T.reshape(NKT, 128, T).transpose(1, 0, 2)
    ).reshape(128, NKT * T)                        # [p, kt*T]
    # per-group token sums (beta part): xsumT[g, t]
    xsumT = np.ascontiguousarray(
        xf.reshape(T, NG, GS).sum(axis=2, dtype=np.float64).T.astype(np.float32)
    )

    shifts = (29 - 3 * np.arange(NF, dtype=np.uint32))[None, None, :, None]
    in_maps = []
    for c in range(NCORES):
        cols = slice(OPC * c, OPC * (c + 1))
        Vc = S[rows.reshape(-1), cols].reshape(NKT, 128, OPC).transpose(1, 0, 2)
        Fb = (Vc ^ 4).astype(np.uint32).reshape(128, NKT, NF, NWC)
        W = (Fb << shifts).sum(axis=2, dtype=np.uint64).astype(np.uint32)
        consts = np.zeros((128, OPC + OPC + T), np.float32)
        consts[:, :OPC] = A2full[np.arange(128) // 4][:, cols]
        consts[:NG, OPC:2 * OPC] = Bfull[:, cols]
        consts[:NG, 2 * OPC:] = xsumT
        in_maps.append(dict(
            xt=XT,
            w=np.ascontiguousarray(W.reshape(128, NKT * NWC)).view(np.int32),
            consts=consts,
        ))
    return in_maps


# ---------------------------------------------------------------- program
def build_program():
    nc = bacc.Bacc("TRN2")
    xt = nc.declare_dram_parameter("xt", [128, NKT * T], F32, isOutput=False)
    w = nc.declare_dram_parameter("w", [128, NKT * NWC], I32, isOutput=False)
    consts = nc.declare_dram_parameter("consts", [128, OPC + OPC + T], F32,
                                       isOutput=False)
    z = nc.declare_dram_parameter("z", [T, OPC], F32, isOutput=True)

    CW = KTC * NWC            # words per chunk per partition

    with tile.TileContext(nc) as tc, ExitStack() as ctx:
        cpool = ctx.enter_context(tc.tile_pool(name="const", bufs=1))
        wmpool = ctx.enter_context(tc.tile_pool(name="wm", bufs=6))
        opool = ctx.enter_context(tc.tile_pool(name="out", bufs=1))
        ppool = ctx.enter_context(tc.tile_pool(name="psum", bufs=1, space="PSUM"))

        cst_sb = cpool.tile([128, OPC + OPC + T], F32, tag="cst")
        nc.sync.dma_start(out=cst_sb[:], in_=consts[:])
        a2_sb = cst_sb[:, :OPC]
        bm_sb = cst_sb[:NG, OPC:2 * OPC]
        xs_sb = cst_sb[:NG, 2 * OPC:]

        w_sb = cpool.tile([128, NKT * NWC], I32, tag="w")
        nc.sync.dma_start(out=w_sb[:], in_=w[:])
        xt_sb = cpool.tile([128, NKT * T], F32, tag="xt")
        nc.sync.dma_start(out=xt_sb[:], in_=xt[:])

        v_sb = [cpool.tile([128, NF * CW], I32, tag=f"v{ch}", name=f"v{ch}")
                for ch in range(NCHUNK)]

        psum_main = ppool.tile([T, OPC], F32, tag="main")
        for ch in range(NCHUNK):
            # unpack: V[p, r*CW + kt*64 + c] = (W-chunk << 3r) >>a 29
            for r in range(NF):
                nc.vector.tensor_scalar(
                    v_sb[ch][:, r * CW:(r + 1) * CW],
                    w_sb[:, ch * CW:(ch + 1) * CW],
                    3 * r,
                    29,
                    ALU.logical_shift_left,
                    ALU.arith_shift_right,
                )
            v4 = v_sb[ch][:].rearrange("p (r kt c) -> p kt r c", r=NF, kt=KTC, c=NWC)
            for k in range(KTC):
                kt = ch * KTC + k
                wm = wmpool.tile([128, OPC], F32, tag="wm")
                nc.vector.tensor_tensor(
                    wm[:].rearrange("p (r c) -> p r c", r=NF),
                    v4[:, k],
                    a2_sb.rearrange("p (r c) -> p r c", r=NF),
                    ALU.mult,
                )
                nc.tensor.matmul(
                    psum_main[:],
                    xt_sb[:, kt * T:(kt + 1) * T],
                    wm[:],
                    start=(kt == 0),
                    stop=False,
                )
        nc.tensor.matmul(psum_main[:], xs_sb, bm_sb, start=False, stop=True)

        out_sb = opool.tile([T, OPC], F32, tag="out_sb")
        nc.vector.tensor_copy(out_sb[:], psum_main[:])
        nc.sync.dma_start(out=z[:], in_=out_sb[:])
    nc.finalize()
    return nc


def _get_program():
    if "nc" not in _PROGRAM_CACHE:
        _PROGRAM_CACHE["nc"] = build_program()
    return _PROGRAM_CACHE["nc"]


# ---------------------------------------------------------------- entry
def kernel(**inputs):
    from concourse.bass_utils import run_bass_kernel_spmd

    in_maps = _prepare(inputs)
    nc = _get_program()
    res = run_bass_kernel_spmd(nc, in_maps, list(range(NCORES)))
    z = np.concatenate([res.results[c]["z"] for c in range(NCORES)], axis=1)
    out_reorder = np.asarray(inputs["out_reorder"], np.int64)
    y = z[:, out_reorder].reshape(1, T, OUT_F).astype(np.float32)
    return y


# revision 17
# speedup vs baseline: 1.6534x; 1.6534x over previous
"""BCQLinear (3-bit binary-coded quantized linear) Trainium2 kernel.

Full-input contract: kernel(**inputs) takes the unsharded inputs of
nn_BCQLinear_88510686036218 and returns the full [1, 128, 4096] output.

Math: w = alpha*(2*S-7) + beta with S in [0,8) the 3-bit code, then
y = (x[:, in_reorder] @ w)[:, out_reorder].
Rewritten: w = A2*S + B with A2 = 2*alpha, B = beta - 7*alpha.

Sharding: out-features split 8 ways (512 cols/core), x replicated.

Per-core device program (SPMD, one Bass program):
  - Contraction rows are band-packed: K-tile kt in [0,32), partition p:
    dequant row i(kt,p) = 128*(p//4) + 4*kt + (p%4), so a single [128,512]
    alpha tile (A2_rep[p,:] = A2[p//4,:]) serves every K-tile.
  - Codes arrive as packed int32 words holding 4 PAIRS of 3-bit fields:
    pair r at bits [3r,3r+3) (lo) and [16+3r,16+3r+3) (hi).  One two-op
    tensor_scalar (>>3r, &0x70007) extracts a pair per element; the
    result buffer reinterpreted as int16 is a dense stream of codes, so
    the dequant multiply (codes x fp16 alpha) runs in the DVE 16-bit
    2x perf mode, and the matmuls run fp16 (full PE rate, fp32 PSUM).
  - The beta part is y += xsum @ B with xsum[g,t] per-group sums of
    permuted x (host-computed; 0.01% of the FLOPs) via one K=32 matmul.
"""
import numpy as np
from contextlib import ExitStack

import concourse.bass as bass
import concourse.mybir as mybir
import concourse.tile as tile
from concourse import bacc

IN_F, OUT_F, WBITS, GS, OFI = 4096, 4096, 3, 128, 128
NG, NB = 32, 32
NCORES = 8
OPC = OUT_F // NCORES        # 512 out-cols per core
NKT = 32                     # K-tiles of 128 rows
NR = 4                       # field-pairs per int32 word
NWC = OPC // (2 * NR)        # 64 packed words per (partition, K-tile)
T = 128                      # tokens
NCHUNK = 2                   # pipeline chunks
KTC = NKT // NCHUNK          # K-tiles per chunk

F32 = mybir.dt.float32
F16 = mybir.dt.float16
I32 = mybir.dt.int32
I16 = mybir.dt.int16
ALU = mybir.AluOpType

_PROGRAM_CACHE = {}


# ---------------------------------------------------------------- host prep
def _dequant_codes(qweight):
    """S[i, o] in [0,8): w = alpha*(2S-7)+beta."""
    qw = np.asarray(qweight, dtype=np.uint32).reshape(NG, NB, WBITS, GS * OFI // 32)
    bits = (qw[..., None] >> np.arange(32, dtype=np.uint32)) & 1
    bits = bits.reshape(NG, NB, WBITS, GS, OFI)
    S = (bits * (2 ** np.arange(WBITS, dtype=np.uint32))[:, None, None]).sum(axis=2)
    return S.transpose(0, 2, 1, 3).reshape(IN_F, OUT_F).astype(np.uint32)


def _band_rows():
    kt, p = np.meshgrid(np.arange(NKT), np.arange(128), indexing="ij")
    return 128 * (p // 4) + 4 * kt + (p % 4)      # [NKT, 128]


def _prepare(inputs):
    x = np.asarray(inputs["x"], np.float32).reshape(-1, IN_F)
    alpha = np.asarray(inputs["alpha"], np.float32)
    beta = np.asarray(inputs["beta"], np.float32)
    in_reorder = np.asarray(inputs["in_reorder"], np.int64)
    xf = x[:, in_reorder]

    S = _dequant_codes(inputs["qweight"])          # [IN_F, OUT_F] uint32
    A2full = (2.0 * alpha).astype(np.float16)
    Bfull = (beta.astype(np.float64) - 7.0 * alpha.astype(np.float64)
             ).astype(np.float16)

    rows = _band_rows()                            # [NKT, 128]
    XT = np.ascontiguousarray(
        xf[:, rows.reshape(-1)].T.reshape(NKT, 128, T).transpose(1, 0, 2)
    ).reshape(128, NKT * T).astype(np.float16)     # [p, kt*T]
    # per-group token sums (beta part): xsumT[g, t]
    xsumT = (xf.reshape(T, NG, GS).sum(axis=2, dtype=np.float64)
             .T.astype(np.float16))

    in_maps = []
    for c in range(NCORES):
        cols = slice(OPC * c, OPC * (c + 1))
        # codes for this core in banded row order: [p, kt, o']
        Sc = S[rows.reshape(-1), cols].reshape(NKT, 128, OPC).transpose(1, 0, 2)
        # pack pairs: o' = r*128 + 2c' + h  ->  bits [3r+16h, +3)
        W = np.zeros((128, NKT, NWC), np.uint32)
        for r in range(NR):
            for h in range(2):
                W |= Sc[:, :, r * 128 + h::2][:, :, :NWC] << (3 * r + 16 * h)
        consts = np.zeros((128, OPC + OPC + T), np.float16)
        consts[:, :OPC] = A2full[np.arange(128) // 4][:, cols]
        consts[:NG, OPC:2 * OPC] = Bfull[:, cols]
        consts[:NG, 2 * OPC:] = xsumT
        in_maps.append(dict(
            xt=XT,
            w=np.ascontiguousarray(W.reshape(128, NKT * NWC)).view(np.int32),
            consts=consts,
        ))
    return in_maps


# ---------------------------------------------------------------- program
def build_program():
    nc = bacc.Bacc("TRN2")
    xt = nc.declare_dram_parameter("xt", [128, NKT * T], F16, isOutput=False)
    w = nc.declare_dram_parameter("w", [128, NKT * NWC], I32, isOutput=False)
    consts = nc.declare_dram_parameter("consts", [128, OPC + OPC + T], F16,
                                       isOutput=False)
    z = nc.declare_dram_parameter("z", [T, OPC], F32, isOutput=True)

    CW = KTC * NWC            # packed words per chunk per partition

    with tile.TileContext(nc) as tc, ExitStack() as ctx:
        cpool = ctx.enter_context(tc.tile_pool(name="const", bufs=1))
        wmpool = ctx.enter_context(tc.tile_pool(name="wm", bufs=6))
        opool = ctx.enter_context(tc.tile_pool(name="out", bufs=1))
        ppool = ctx.enter_context(tc.tile_pool(name="psum", bufs=1, space="PSUM"))

        cst_sb = cpool.tile([128, OPC + OPC + T], F16, tag="cst")
        nc.sync.dma_start(out=cst_sb[:], in_=consts[:])
        a2_sb = cst_sb[:, :OPC]
        bm_sb = cst_sb[:NG, OPC:2 * OPC]
        xs_sb = cst_sb[:NG, 2 * OPC:]

        w_sb = cpool.tile([128, NKT * NWC], I32, tag="w")
        for ch in range(NCHUNK):
            nc.sync.dma_start(out=w_sb[:, ch * CW:(ch + 1) * CW],
                              in_=w[:, ch * CW:(ch + 1) * CW])
        xt_sb = cpool.tile([128, NKT * T], F16, tag="xt")
        nc.sync.dma_start(out=xt_sb[:], in_=xt[:])

        v_sb = [cpool.tile([128, NR * CW], I32, tag=f"v{ch}", name=f"v{ch}")
                for ch in range(NCHUNK)]

        psum_main = ppool.tile([T, OPC], F32, tag="main")
        for ch in range(NCHUNK):
            # unpack pair r: V32[p, r*CW + k*NWC + c] = (W >> 3r) & 0x70007
            for r in range(NR):
                nc.vector.tensor_scalar(
                    v_sb[ch][:, r * CW:(r + 1) * CW],
                    w_sb[:, ch * CW:(ch + 1) * CW],
                    3 * r,
                    0x00070007,
                    ALU.logical_shift_right,
                    ALU.bitwise_and,
                )
            # int16 view: [p, (r, k, q)] with q = 2c+h in [0,256), o' = r*128+q
            v16 = v_sb[ch][:].bitcast(I16).rearrange(
                "p (r k q) -> p k r q", r=NR, k=KTC, q=2 * NWC)
            for k in range(KTC):
                kt = ch * KTC + k
                wm = wmpool.tile([128, OPC], F16, tag="wm")
                nc.vector.tensor_tensor(
                    wm[:].rearrange("p (r q) -> p r q", r=NR),
                    v16[:, k],
                    a2_sb.rearrange("p (r q) -> p r q", r=NR),
                    ALU.mult,
                )
                nc.tensor.matmul(
                    psum_main[:],
                    xt_sb[:, kt * T:(kt + 1) * T],
                    wm[:],
                    start=(kt == 0),
                    stop=False,
                )
        nc.tensor.matmul(psum_main[:], xs_sb, bm_sb, start=False, stop=True)

        out_sb = opool.tile([T, OPC], F32, tag="out_sb")
        nc.vector.tensor_copy(out_sb[:], psum_main[:])
        nc.sync.dma_start(out=z[:], in_=out_sb[:])
    nc.finalize()
    return nc


def _get_program():
    if "nc" not in _PROGRAM_CACHE:
        _PROGRAM_CACHE["nc"] = build_program()
    return _PROGRAM_CACHE["nc"]


# ---------------------------------------------------------------- entry
def kernel(**inputs):
    from concourse.bass_utils import run_bass_kernel_spmd

    in_maps = _prepare(inputs)
    nc = _get_program()
    res = run_bass_kernel_spmd(nc, in_maps, list(range(NCORES)))
    z = np.concatenate([res.results[c]["z"] for c in range(NCORES)], axis=1)
    out_reorder = np.asarray(inputs["out_reorder"], np.int64)
    y = z[:, out_reorder].reshape(1, T, OUT_F).astype(np.float32)
    return y


# revision 18
# speedup vs baseline: 1.7239x; 1.0427x over previous
"""BCQLinear (3-bit binary-coded quantized linear) Trainium2 kernel.

Full-input contract: kernel(**inputs) takes the unsharded inputs of
nn_BCQLinear_88510686036218 and returns the full [1, 128, 4096] output.

Math: w = alpha*(2*S-7) + beta with S in [0,8) the 3-bit code, then
y = (x[:, in_reorder] @ w)[:, out_reorder].
Rewritten: w = A2*S + B with A2 = 2*alpha, B = beta - 7*alpha.

Sharding: out-features split 8 ways (512 cols/core), x replicated.

Per-core device program (SPMD, one Bass program):
  - Contraction rows are band-packed: K-tile kt in [0,32), partition p:
    dequant row i(kt,p) = 128*(p//4) + 4*kt + (p%4), so a single [128,512]
    alpha tile (A2_rep[p,:] = A2[p//4,:]) serves every K-tile.
  - Codes arrive as packed int32 words holding 4 PAIRS of 3-bit fields:
    pair r at bits [3r,3r+3) (lo) and [16+3r,16+3r+3) (hi).  One two-op
    tensor_scalar (>>3r, &0x70007) extracts a pair per element; the
    result buffer reinterpreted as int16 is a dense stream of codes, so
    the dequant multiply (codes x fp16 alpha) runs in the DVE 16-bit
    2x perf mode, and the matmuls run fp16 (full PE rate, fp32 PSUM).
  - The beta part is y += xsum @ B with xsum[g,t] per-group sums of
    permuted x (host-computed; 0.01% of the FLOPs) via one fp32 K=32
    matmul into the same PSUM accumulation.
  - W streams on the sync queue in quarters (unpack starts after the
    first quarter); alpha / x / fp32 consts stream on the scalar queue.
"""
import numpy as np
from contextlib import ExitStack

import concourse.bass as bass
import concourse.mybir as mybir
import concourse.tile as tile
from concourse import bacc

IN_F, OUT_F, WBITS, GS, OFI = 4096, 4096, 3, 128, 128
NG, NB = 32, 32
NCORES = 8
OPC = OUT_F // NCORES        # 512 out-cols per core
NKT = 32                     # K-tiles of 128 rows
NR = 4                       # field-pairs per int32 word
NWC = OPC // (2 * NR)        # 64 packed words per (partition, K-tile)
T = 128                      # tokens
NCHUNK = 4                   # w/unpack pipeline chunks
KTC = NKT // NCHUNK          # K-tiles per chunk
NBAT = 2                     # K-tiles per dequant-multiply batch

F32 = mybir.dt.float32
F16 = mybir.dt.float16
I32 = mybir.dt.int32
I16 = mybir.dt.int16
ALU = mybir.AluOpType

_PROGRAM_CACHE = {}


# ---------------------------------------------------------------- host prep
def _dequant_codes(qweight):
    """S[i, o] in [0,8): w = alpha*(2S-7)+beta."""
    qw = np.asarray(qweight, dtype=np.uint32).reshape(NG, NB, WBITS, GS * OFI // 32)
    bits = (qw[..., None] >> np.arange(32, dtype=np.uint32)) & 1
    bits = bits.reshape(NG, NB, WBITS, GS, OFI)
    S = (bits * (2 ** np.arange(WBITS, dtype=np.uint32))[:, None, None]).sum(axis=2)
    return S.transpose(0, 2, 1, 3).reshape(IN_F, OUT_F).astype(np.uint32)


def _band_rows():
    kt, p = np.meshgrid(np.arange(NKT), np.arange(128), indexing="ij")
    return 128 * (p // 4) + 4 * kt + (p % 4)      # [NKT, 128]


def _prepare(inputs):
    x = np.asarray(inputs["x"], np.float32).reshape(-1, IN_F)
    alpha = np.asarray(inputs["alpha"], np.float32)
    beta = np.asarray(inputs["beta"], np.float32)
    in_reorder = np.asarray(inputs["in_reorder"], np.int64)
    xf = x[:, in_reorder]

    S = _dequant_codes(inputs["qweight"])          # [IN_F, OUT_F] uint32
    A2full = (2.0 * alpha).astype(np.float16)
    Bfull = (beta.astype(np.float64) - 7.0 * alpha.astype(np.float64)
             ).astype(np.float32)

    rows = _band_rows()                            # [NKT, 128]
    XT = np.ascontiguousarray(
        xf[:, rows.reshape(-1)].T.reshape(NKT, 128, T).transpose(1, 0, 2)
    ).reshape(128, NKT * T).astype(np.float16)     # [p, kt*T]
    # per-group token sums (beta part): xsumT[g, t]
    xsumT = (xf.reshape(T, NG, GS).sum(axis=2, dtype=np.float64)
             .T.astype(np.float32))

    in_maps = []
    for c in range(NCORES):
        cols = slice(OPC * c, OPC * (c + 1))
        # codes for this core in banded row order: [p, kt, o']
        Sc = S[rows.reshape(-1), cols].reshape(NKT, 128, OPC).transpose(1, 0, 2)
        # pack pairs: o' = r*128 + 2c' + h  ->  bits [3r+16h, +3)
        W = np.zeros((128, NKT, NWC), np.uint32)
        for r in range(NR):
            for h in range(2):
                W |= Sc[:, :, r * 128 + h::2][:, :, :NWC] << (3 * r + 16 * h)
        a2rep = A2full[np.arange(128) // 4][:, cols]
        consts32 = np.zeros((NG, OPC + T), np.float32)
        consts32[:, :OPC] = Bfull[:, cols]
        consts32[:, OPC:] = xsumT
        in_maps.append(dict(
            xt=XT,
            w=np.ascontiguousarray(W.reshape(128, NKT * NWC)).view(np.int32),
            a2dup=np.ascontiguousarray(np.tile(a2rep, (1, NBAT))),
            consts32=consts32,
        ))
    return in_maps


# ---------------------------------------------------------------- program
def build_program():
    nc = bacc.Bacc("TRN2")
    xt = nc.declare_dram_parameter("xt", [128, NKT * T], F16, isOutput=False)
    w = nc.declare_dram_parameter("w", [128, NKT * NWC], I32, isOutput=False)
    a2dup = nc.declare_dram_parameter("a2dup", [128, NBAT * OPC], F16,
                                      isOutput=False)
    consts32 = nc.declare_dram_parameter("consts32", [NG, OPC + T], F32,
                                         isOutput=False)
    z = nc.declare_dram_parameter("z", [T, OPC], F32, isOutput=True)

    CW = KTC * NWC            # packed words per chunk per partition
    XQ = NKT * T // 4         # xt quarter width

    with tile.TileContext(nc) as tc, ExitStack() as ctx:
        cpool = ctx.enter_context(tc.tile_pool(name="const", bufs=1))
        wmpool = ctx.enter_context(tc.tile_pool(name="wm", bufs=4))
        opool = ctx.enter_context(tc.tile_pool(name="out", bufs=1))
        ppool = ctx.enter_context(tc.tile_pool(name="psum", bufs=1, space="PSUM"))

        # sync queue: the packed codes, in quarters
        w_sb = cpool.tile([128, NKT * NWC], I32, tag="w")
        for ch in range(NCHUNK):
            nc.sync.dma_start(out=w_sb[:, ch * CW:(ch + 1) * CW],
                              in_=w[:, ch * CW:(ch + 1) * CW])
        # scalar queue: alpha, then x in quarters, then fp32 consts
        a2_sb = cpool.tile([128, NBAT * OPC], F16, tag="a2")
        nc.scalar.dma_start(out=a2_sb[:], in_=a2dup[:])
        xt_sb = cpool.tile([128, NKT * T], F16, tag="xt")
        for q in range(4):
            nc.scalar.dma_start(out=xt_sb[:, q * XQ:(q + 1) * XQ],
                                in_=xt[:, q * XQ:(q + 1) * XQ])
        c32_sb = cpool.tile([NG, OPC + T], F32, tag="c32")
        nc.scalar.dma_start(out=c32_sb[:], in_=consts32[:])
        bm_sb = c32_sb[:, :OPC]
        xs_sb = c32_sb[:, OPC:]

        v_sb = [cpool.tile([128, NR * CW], I32, tag=f"v{ch}", name=f"v{ch}")
                for ch in range(NCHUNK)]

        psum_main = ppool.tile([T, OPC], F32, tag="main")
        for ch in range(NCHUNK):
            # unpack pair r: V32[p, r*CW + k*NWC + c] = (W >> 3r) & 0x70007
            for r in range(NR):
                nc.vector.tensor_scalar(
                    v_sb[ch][:, r * CW:(r + 1) * CW],
                    w_sb[:, ch * CW:(ch + 1) * CW],
                    3 * r,
                    0x00070007,
                    ALU.logical_shift_right,
                    ALU.bitwise_and,
                )
            # int16 view: [p, (r, k, q)], q = 2c+h in [0,256), o' = r*128+q
            v16 = v_sb[ch][:].bitcast(I16).rearrange(
                "p (r k q) -> p k r q", r=NR, k=KTC, q=2 * NWC)
            for b in range(KTC // NBAT):
                # dequant-multiply NBAT K-tiles in one 16-bit 2x-mode op
                wm = wmpool.tile([128, NBAT * OPC], F16, tag="wm")
                nc.vector.tensor_tensor(
                    wm[:].rearrange("p (k r q) -> p k r q", k=NBAT, r=NR),
                    v16[:, b * NBAT:(b + 1) * NBAT],
                    a2_sb[:].rearrange("p (k r q) -> p k r q", k=NBAT, r=NR),
                    ALU.mult,
                )
                for j in range(NBAT):
                    kt = ch * KTC + b * NBAT + j
                    nc.tensor.matmul(
                        psum_main[:],
                        xt_sb[:, kt * T:(kt + 1) * T],
                        wm[:, j * OPC:(j + 1) * OPC],
                        start=(kt == 0),
                        stop=False,
                    )
        nc.tensor.matmul(psum_main[:], xs_sb, bm_sb, start=False, stop=True)

        out_sb = opool.tile([T, OPC], F32, tag="out_sb")
        nc.scalar.copy(out=out_sb[:], in_=psum_main[:])
        nc.sync.dma_start(out=z[:], in_=out_sb[:])
    nc.finalize()
    return nc


def _get_program():
    if "nc" not in _PROGRAM_CACHE:
        _PROGRAM_CACHE["nc"] = build_program()
    return _PROGRAM_CACHE["nc"]


# ---------------------------------------------------------------- entry
def kernel(**inputs):
    from concourse.bass_utils import run_bass_kernel_spmd

    in_maps = _prepare(inputs)
    nc = _get_program()
    res = run_bass_kernel_spmd(nc, in_maps, list(range(NCORES)))
    z = np.concatenate([res.results[c]["z"] for c in range(NCORES)], axis=1)
    out_reorder = np.asarray(inputs["out_reorder"], np.int64)
    y = z[:, out_reorder].reshape(1, T, OUT_F).astype(np.float32)
    return y


# revision 19
# speedup vs baseline: 1.7561x; 1.0187x over previous
"""BCQLinear (3-bit binary-coded quantized linear) Trainium2 kernel.

Full-input contract: kernel(**inputs) takes the unsharded inputs of
nn_BCQLinear_88510686036218 and returns the full [1, 128, 4096] output.

Math: w = alpha*(2*S-7) + beta with S in [0,8) the 3-bit code, then
y = (x[:, in_reorder] @ w)[:, out_reorder].
Rewritten: w = A2*S + B with A2 = 2*alpha, B = beta - 7*alpha.

Sharding: out-features split 8 ways (512 cols/core), x replicated.

Per-core device program (SPMD, one Bass program):
  - Contraction rows are band-packed: K-tile kt in [0,32), partition p:
    dequant row i(kt,p) = 128*(p//4) + 4*kt + (p%4), so a single [128,512]
    alpha tile (A2_rep[p,:] = A2[p//4,:]) serves every K-tile.
  - Codes arrive as packed int32 words holding 4 PAIRS of 3-bit fields:
    pair r at bits [3r,3r+3) (lo) and [16+3r,16+3r+3) (hi).  One two-op
    tensor_scalar (>>3r, &0x70007) extracts a pair per element; the
    result buffer reinterpreted as int16 is a dense stream of codes, so
    the dequant multiply (codes x fp16 alpha) runs in the DVE 16-bit
    2x perf mode, and the matmuls run fp16 (full PE rate, fp32 PSUM).
  - The beta part is y += xsum @ B with xsum[g,t] per-group sums of
    permuted x (host-computed; 0.01% of the FLOPs) via one fp32 K=32
    matmul into the same PSUM accumulation.
  - W and X stream as per-chunk contiguous DRAM tensors on two DMA
    queues (sync + scalar); a short warm-up matmul burst runs during the
    DMA window so the PE HAM clock-gate is released before real work.
"""
import numpy as np
from contextlib import ExitStack

import concourse.bass as bass
import concourse.mybir as mybir
import concourse.tile as tile
from concourse import bacc

IN_F, OUT_F, WBITS, GS, OFI = 4096, 4096, 3, 128, 128
NG, NB = 32, 32
NCORES = 8
OPC = OUT_F // NCORES        # 512 out-cols per core
NKT = 32                     # K-tiles of 128 rows
NR = 4                       # field-pairs per int32 word
NWC = OPC // (2 * NR)        # 64 packed words per (partition, K-tile)
T = 128                      # tokens
NCHUNK = 4                   # w/unpack pipeline chunks
KTC = NKT // NCHUNK          # K-tiles per chunk
NBAT = 2                     # K-tiles per dequant-multiply batch
NWARM = 14                   # PE warm-up matmuls

F32 = mybir.dt.float32
F16 = mybir.dt.float16
I32 = mybir.dt.int32
I16 = mybir.dt.int16
ALU = mybir.AluOpType

_PROGRAM_CACHE = {}


# ---------------------------------------------------------------- host prep
def _dequant_codes(qweight):
    """S[i, o] in [0,8): w = alpha*(2S-7)+beta."""
    qw = np.asarray(qweight, dtype=np.uint32).reshape(NG, NB, WBITS, GS * OFI // 32)
    bits = (qw[..., None] >> np.arange(32, dtype=np.uint32)) & 1
    bits = bits.reshape(NG, NB, WBITS, GS, OFI)
    S = (bits * (2 ** np.arange(WBITS, dtype=np.uint32))[:, None, None]).sum(axis=2)
    return S.transpose(0, 2, 1, 3).reshape(IN_F, OUT_F).astype(np.uint32)


def _band_rows():
    kt, p = np.meshgrid(np.arange(NKT), np.arange(128), indexing="ij")
    return 128 * (p // 4) + 4 * kt + (p % 4)      # [NKT, 128]


def _prepare(inputs):
    x = np.asarray(inputs["x"], np.float32).reshape(-1, IN_F)
    alpha = np.asarray(inputs["alpha"], np.float32)
    beta = np.asarray(inputs["beta"], np.float32)
    in_reorder = np.asarray(inputs["in_reorder"], np.int64)
    xf = x[:, in_reorder]

    S = _dequant_codes(inputs["qweight"])          # [IN_F, OUT_F] uint32
    A2full = (2.0 * alpha).astype(np.float16)
    Bfull = (beta.astype(np.float64) - 7.0 * alpha.astype(np.float64)
             ).astype(np.float32)

    rows = _band_rows()                            # [NKT, 128]
    XT = np.ascontiguousarray(
        xf[:, rows.reshape(-1)].T.reshape(NKT, 128, T).transpose(1, 0, 2)
    ).reshape(128, NKT * T).astype(np.float16)     # [p, kt*T]
    # per-group token sums (beta part): xsumT[g, t]
    xsumT = (xf.reshape(T, NG, GS).sum(axis=2, dtype=np.float64)
             .T.astype(np.float32))

    CW = KTC * NWC
    XQ = NKT * T // NCHUNK
    in_maps = []
    for c in range(NCORES):
        cols = slice(OPC * c, OPC * (c + 1))
        # codes for this core in banded row order: [p, kt, o']
        Sc = S[rows.reshape(-1), cols].reshape(NKT, 128, OPC).transpose(1, 0, 2)
        # pack pairs: o' = r*128 + 2c' + h  ->  bits [3r+16h, +3)
        W = np.zeros((128, NKT, NWC), np.uint32)
        for r in range(NR):
            for h in range(2):
                W |= Sc[:, :, r * 128 + h::2][:, :, :NWC] << (3 * r + 16 * h)
        W = W.reshape(128, NKT * NWC)
        a2rep = A2full[np.arange(128) // 4][:, cols]
        consts32 = np.zeros((NG, OPC + T), np.float32)
        consts32[:, :OPC] = Bfull[:, cols]
        consts32[:, OPC:] = xsumT
        im = dict(
            a2dup=np.ascontiguousarray(np.tile(a2rep, (1, NBAT))),
            consts32=consts32,
        )
        for ch in range(NCHUNK):
            im[f"w{ch}"] = np.ascontiguousarray(
                W[:, ch * CW:(ch + 1) * CW]).view(np.int32)
            im[f"xt{ch}"] = np.ascontiguousarray(XT[:, ch * XQ:(ch + 1) * XQ])
        in_maps.append(im)
    return in_maps


# ---------------------------------------------------------------- program
def build_program():
    nc = bacc.Bacc("TRN2")
    CW = KTC * NWC            # packed words per chunk per partition
    XQ = NKT * T // NCHUNK    # xt columns per chunk

    w_dr = [nc.declare_dram_parameter(f"w{ch}", [128, CW], I32, isOutput=False)
            for ch in range(NCHUNK)]
    xt_dr = [nc.declare_dram_parameter(f"xt{ch}", [128, XQ], F16, isOutput=False)
             for ch in range(NCHUNK)]
    a2dup = nc.declare_dram_parameter("a2dup", [128, NBAT * OPC], F16,
                                      isOutput=False)
    consts32 = nc.declare_dram_parameter("consts32", [NG, OPC + T], F32,
                                         isOutput=False)
    z = nc.declare_dram_parameter("z", [T, OPC], F32, isOutput=True)

    with tile.TileContext(nc) as tc, ExitStack() as ctx:
        cpool = ctx.enter_context(tc.tile_pool(name="const", bufs=1))
        wmpool = ctx.enter_context(tc.tile_pool(name="wm", bufs=4))
        opool = ctx.enter_context(tc.tile_pool(name="out", bufs=1))
        ppool = ctx.enter_context(tc.tile_pool(name="psum", bufs=1, space="PSUM"))

        # sync queue: packed code chunks
        w_sb = []
        for ch in range(NCHUNK):
            wt = cpool.tile([128, CW], I32, tag=f"w{ch}", name=f"wsb{ch}")
            nc.sync.dma_start(out=wt[:], in_=w_dr[ch][:])
            w_sb.append(wt)
        # scalar queue: alpha, then x chunks, then fp32 consts
        a2_sb = cpool.tile([128, NBAT * OPC], F16, tag="a2")
        nc.scalar.dma_start(out=a2_sb[:], in_=a2dup[:])
        xt_sb = []
        for ch in range(NCHUNK):
            xtt = cpool.tile([128, XQ], F16, tag=f"xt{ch}", name=f"xtsb{ch}")
            nc.scalar.dma_start(out=xtt[:], in_=xt_dr[ch][:])
            xt_sb.append(xtt)
        c32_sb = cpool.tile([NG, OPC + T], F32, tag="c32")
        nc.scalar.dma_start(out=c32_sb[:], in_=consts32[:])
        bm_sb = c32_sb[:, :OPC]
        xs_sb = c32_sb[:, OPC:]

        # PE warm-up: garbage matmuls on a memset tile into a scratch bank
        wu_sb = cpool.tile([128, OPC], F16, tag="wu")
        nc.gpsimd.memset(wu_sb[:], 0.0)
        psum_wu = ppool.tile([128, OPC], F32, tag="wu_ps")
        for i in range(NWARM):
            nc.tensor.matmul(psum_wu[:], wu_sb[:, :T], wu_sb[:],
                             start=True, stop=True)

        v_sb = [cpool.tile([128, NR * CW], I32, tag=f"v{ch}", name=f"v{ch}")
                for ch in range(NCHUNK)]

        psum_main = ppool.tile([T, OPC], F32, tag="main")
        for ch in range(NCHUNK):
            # unpack pair r: V32[p, r*CW + k*NWC + c] = (W >> 3r) & 0x70007
            for r in range(NR):
                nc.vector.tensor_scalar(
                    v_sb[ch][:, r * CW:(r + 1) * CW],
                    w_sb[ch][:],
                    3 * r,
                    0x00070007,
                    ALU.logical_shift_right,
                    ALU.bitwise_and,
                )
            # int16 view: [p, (r, k, q)], q = 2c+h in [0,256), o' = r*128+q
            v16 = v_sb[ch][:].bitcast(I16).rearrange(
                "p (r k q) -> p k r q", r=NR, k=KTC, q=2 * NWC)
            for b in range(KTC // NBAT):
                # dequant-multiply NBAT K-tiles in one 16-bit 2x-mode op
                wm = wmpool.tile([128, NBAT * OPC], F16, tag="wm")
                nc.vector.tensor_tensor(
                    wm[:].rearrange("p (k r q) -> p k r q", k=NBAT, r=NR),
                    v16[:, b * NBAT:(b + 1) * NBAT],
                    a2_sb[:].rearrange("p (k r q) -> p k r q", k=NBAT, r=NR),
                    ALU.mult,
                )
                for j in range(NBAT):
                    kt = ch * KTC + b * NBAT + j
                    nc.tensor.matmul(
                        psum_main[:],
                        xt_sb[ch][:, (b * NBAT + j) * T:(b * NBAT + j + 1) * T],
                        wm[:, j * OPC:(j + 1) * OPC],
                        start=(kt == 0),
                        stop=False,
                    )
        nc.tensor.matmul(psum_main[:], xs_sb, bm_sb, start=False, stop=True)

        out_sb = opool.tile([T, OPC], F32, tag="out_sb")
        nc.scalar.copy(out=out_sb[:], in_=psum_main[:])
        nc.sync.dma_start(out=z[:], in_=out_sb[:])
    nc.finalize()
    return nc


def _get_program():
    if "nc" not in _PROGRAM_CACHE:
        _PROGRAM_CACHE["nc"] = build_program()
    return _PROGRAM_CACHE["nc"]


# ---------------------------------------------------------------- entry
def kernel(**inputs):
    from concourse.bass_utils import run_bass_kernel_spmd

    in_maps = _prepare(inputs)
    nc = _get_program()
    res = run_bass_kernel_spmd(nc, in_maps, list(range(NCORES)))
    z = np.concatenate([res.results[c]["z"] for c in range(NCORES)], axis=1)
    out_reorder = np.asarray(inputs["out_reorder"], np.int64)
    y = z[:, out_reorder].reshape(1, T, OUT_F).astype(np.float32)
    return y
